# revision 28
# baseline (speedup 1.0000x reference)
"""Trainium2 kernel for nn_Eq2Net_7859790151696.

Device (8 NeuronCores, SPMD, t-sharded, ONE fused dispatch per call): the
action head la = s @ W_action plus its per-(t,b) softmax-at-the-taken-action,
returning e[t,b] = softmax(la[t,b,:])[act[t]] directly (66 KB bf16 readback
instead of 1.4 MB of raw logits). Each core owns 256 of the 2048 steps.
Payload minimization (every tunnel byte is ~7-20 us): s ships as e5m2 fp8
(1.05 MB), W_action as e4m3 fp8 x16 sharded 64 rows/core and AllGathered on
device (147 KB total), the one-hot action mask as bf16. The dispatch is a
cached fast-path jax.jit around the bass_exec primitive -- run_bass_kernel_spmd
re-traces a fresh closure per call (~150 ms); this is the same execution path
(verified bit-exact) without the re-trace.

Host (fully hidden under the device call's ~65 ms in-flight time): the small
stop/start heads (s_i @ [W_stop|W_start], 2049x48) and the strictly sequential
T=2048, B=16 HMM recurrence, reformulated as a chunked linear solve on the
rank-16 structure (O(T*B^2) block forward substitution, ~6 ms; validated
~1e-6 rel err vs the jax reference). Everything e-independent runs while the
device call is in flight; after readback only
tot = sum_t log((e*G[t]).sum()) + const remains (<1 ms).
"""
import numpy as np

T, S, B, A = 2048, 512, 16, 18
PEN = 0.5
NCORES = 8
RPC = 256            # rows (timesteps) per core; 8*256 = T
L, NCHUNK = 128, 16

_prog = None
_dispatch = None
_casts = None


def _ensure_casts():
    """One jax-cpu jitted prep for all device inputs: fp8 casts + one-hot
    build fused in a single dispatch. ~3x faster than ml_dtypes' element
    loops (bit-exact with them), and it runs before the device dispatch."""
    global _casts
    if _casts is None:
        import jax
        import jax.numpy as jnp
        cpu = jax.devices("cpu")[0]

        def prep(s, W, act):
            s8 = s.astype(jnp.float8_e5m2)
            W8 = (W * 16.0).astype(jnp.float8_e4m3)
            oh = jnp.zeros((T, A), jnp.bfloat16).at[
                jnp.arange(T), act].set(1.0, unique_indices=True)
            return s8, W8, oh

        _casts = jax.jit(prep, device=cpu)
    return _casts


def _build_program():
    import concourse.tile as tile
    from concourse import bacc, mybir

    f32 = mybir.dt.float32
    bf16 = mybir.dt.bfloat16
    AF = mybir.ActivationFunctionType
    ALU = mybir.AluOpType
    AX = mybir.AxisListType

    from concourse.masks import make_identity
    fp8 = mybir.dt.float8e4
    fp8e5 = mybir.dt.float8e5

    nc = bacc.Bacc("TRN2", target_bir_lowering=False, debug=False,
                   num_devices=NCORES)
    s_in = nc.dram_tensor("s", [RPC, S], fp8e5, kind="ExternalInput")
    # each core uploads only its 64-row slice of W; AllGather rebuilds the
    # full (512, 288) on device -- 147 KB over the tunnel instead of 1.18 MB
    Wsh = nc.dram_tensor("Wsh", [S // NCORES, B * A], fp8,
                         kind="ExternalInput")
    oh = nc.dram_tensor("oh", [RPC, A], bf16, kind="ExternalInput")
    e_out = nc.dram_tensor("e", [RPC, B], bf16, kind="ExternalOutput")

    with tile.TileContext(nc) as tc:
        with tc.tile_pool(name="sb", bufs=1) as pool, \
             tc.tile_pool(name="ps", bufs=2, space="PSUM") as pps, \
             tc.tile_pool(name="dram", bufs=1, space="DRAM") as dram:
            # s arrives row-major as e5m2 (half the wire bytes of bf16; the
            # XBAR DMA-transpose only does 2-byte dtypes, so upcast to bf16
            # first and transpose 128x128 blocks on the PE instead)
            s8 = pool.tile([128, 2, S], fp8e5, tag="s8")
            sb = pool.tile([128, 2, S], bf16, tag="sbm")
            for m in range(2):
                nc.gpsimd.dma_start(s8[:, m, :], s_in[m * 128:(m + 1) * 128, :])
                nc.scalar.copy(sb[:, m, :], s8[:, m, :])
            ident = pool.tile([128, 128], bf16, tag="ident")
            make_identity(nc, ident[:])
            sT_sb = pool.tile([128, 4, RPC], bf16, tag="sT")
            with tc.tile_pool(name="pst", bufs=2, space="PSUM") as ppt:
                for m in range(2):
                    for k in range(4):
                        pt = ppt.tile([128, 128], bf16, tag="pt")
                        nc.tensor.transpose(pt[:],
                                            sb[:, m, k * 128:(k + 1) * 128],
                                            ident[:])
                        nc.scalar.copy(sT_sb[:, k, m * 128:(m + 1) * 128],
                                       pt[:])
            # collectives can't touch I/O tensors; bounce through DRAM tiles
            w_in = dram.tile([S // NCORES, B * A], fp8, tag="win")
            w_all = dram.tile([S, B * A], fp8, tag="wall")
            nc.gpsimd.dma_start(w_in[:], Wsh[:])
            nc.gpsimd.collective_compute(
                "AllGather", mybir.AluOpType.bypass,
                replica_groups=[list(range(NCORES))],
                ins=[w_in.opt()], outs=[w_all.opt()])
            # W ships as e4m3 scaled x16 (keeps it in fp8's normal range);
            # the upcast copy to bf16 folds in the 1/16
            W_sb = pool.tile([128, 4, B * A], bf16, tag="W")
            for k in range(4):
                wr = pool.tile([128, B * A], fp8, tag=f"Wr{k}")
                nc.gpsimd.dma_start(wr[:], w_all[k * 128:(k + 1) * 128, :])
                nc.scalar.mul(W_sb[:, k, :], wr[:], 1.0 / 16.0)
            oh_b = pool.tile([128, 2, A], bf16, tag="ohb")
            oh_sb = pool.tile([128, 2, A], f32, tag="oh")
            for m in range(2):
                nc.gpsimd.dma_start(oh_b[:, m, :], oh[m * 128:(m + 1) * 128, :])
                nc.scalar.copy(oh_sb[:, m, :], oh_b[:, m, :])

            for m in range(2):
                ps = pps.tile([128, B * A], mybir.dt.float32, tag=f"ps{m}")
                for k in range(4):
                    nc.tensor.matmul(ps[:], sT_sb[:, k, m * 128:(m + 1) * 128],
                                     W_sb[:, k, :], start=(k == 0),
                                     stop=(k == 3))
                negmax = pool.tile([128, B], mybir.dt.float32, tag=f"nm{m}")
                sums = pool.tile([128, B], mybir.dt.float32, tag=f"sm{m}")
                selx = pool.tile([128, B], mybir.dt.float32, tag=f"sx{m}")
                ebs = pool.tile([128, B, A], mybir.dt.float32, tag=f"eb{m}")
                ttr = pool.tile([128, B, A], mybir.dt.float32, tag=f"tt{m}")
                for b in range(B):
                    blk = ps[:, A * b:A * (b + 1)]
                    nc.vector.tensor_reduce(negmax[:, b:b + 1], blk,
                                            axis=AX.X, op=ALU.max, negate=True)
                    nc.scalar.activation(ebs[:, b, :], blk, AF.Exp,
                                         bias=negmax[:, b:b + 1],
                                         accum_out=sums[:, b:b + 1])
                    # tensor_tensor_reduce passes CoreSim but dies on HW
                    # through this exec path; split into mult + reduce
                    nc.vector.tensor_tensor(ttr[:, b, :], ebs[:, b, :],
                                            oh_sb[:, m, :], ALU.mult)
                    nc.vector.tensor_reduce(selx[:, b:b + 1], ttr[:, b, :],
                                            axis=AX.X, op=ALU.add)
                inv = pool.tile([128, B], mybir.dt.float32, tag=f"iv{m}")
                nc.vector.reciprocal(inv[:], sums[:])
                eo = pool.tile([128, B], mybir.dt.float32, tag=f"eo{m}")
                nc.vector.tensor_tensor(eo[:], selx[:], inv[:], ALU.mult)
                nc.gpsimd.dma_start(e_out[m * 128:(m + 1) * 128, :], eo[:])
    nc.compile()
    return nc


def _build_dispatch(nc):
    """One cached jax.jit(shard_map) around the bass_exec primitive.

    run_bass_kernel_spmd rebuilds (and re-traces) this closure on every
    call, which costs ~150 ms/call under axon; hoisting it is free speed.
    Execution path (bass_exec custom call on the 8 neuron cores) is
    identical -- verified bit-exact against run_bass_kernel_spmd.
    """
    import jax
    from jax.sharding import Mesh, PartitionSpec
    from jax.experimental.shard_map import shard_map
    from concourse import bass2jax, mybir

    bass2jax.install_neuronx_cc_hook()

    partition_name = (nc.partition_id_tensor.name
                      if nc.partition_id_tensor else None)
    in_names, out_names, out_avals = [], [], []
    for alloc in nc.m.functions[0].allocations:
        if not isinstance(alloc, mybir.MemoryLocationSet):
            continue
        name = alloc.memorylocations[0].name
        if alloc.kind == "ExternalInput":
            if name != partition_name:
                in_names.append(name)
        elif alloc.kind == "ExternalOutput":
            out_avals.append(jax.core.ShapedArray(
                tuple(alloc.tensor_shape), mybir.dt.np(alloc.dtype)))
            out_names.append(name)
    all_in_names = list(in_names) + list(out_names)
    if partition_name is not None:
        all_in_names.append(partition_name)

    def _body(*args):
        operands = list(args)
        if partition_name is not None:
            operands.append(bass2jax.partition_id_tensor())
        return tuple(bass2jax._bass_exec_p.bind(
            *operands,
            out_avals=tuple(out_avals),
            in_names=tuple(all_in_names),
            out_names=tuple(out_names),
            lowering_input_output_aliases=(),
            sim_require_finite=True,
            sim_require_nnan=True,
            nc=nc,
        ))

    n_params = len(in_names)
    n_outs = len(out_avals)
    donate = tuple(range(n_params, n_params + n_outs))
    devices = jax.devices()[:NCORES]
    mesh = Mesh(np.asarray(devices), ("core",))
    in_shapes = []
    for alloc in nc.m.functions[0].allocations:
        if not isinstance(alloc, mybir.MemoryLocationSet):
            continue
        if (alloc.kind == "ExternalInput"
                and alloc.memorylocations[0].name in in_names):
            in_shapes.append(jax.ShapeDtypeStruct(
                (NCORES * alloc.tensor_shape[0], *alloc.tensor_shape[1:]),
                mybir.dt.np(alloc.dtype)))
    zero_shapes = [(NCORES * av.shape[0], *av.shape[1:]) for av in out_avals]
    zero_dtypes = [av.dtype for av in out_avals]
    zero_structs = [jax.ShapeDtypeStruct(s, d)
                    for s, d in zip(zero_shapes, zero_dtypes)]

    def _compile():
        return jax.jit(shard_map(
            _body, mesh=mesh,
            in_specs=(PartitionSpec("core"),) * (n_params + n_outs),
            out_specs=(PartitionSpec("core"),) * n_outs,
            check_rep=False),
            donate_argnums=donate, keep_unused=True,
        ).lower(*in_shapes, *zero_structs).compile()

    # C++ fast-path dispatch (no per-call effects machinery)
    sharded = bass2jax.fast_dispatch_compile(_compile)
    return sharded, in_names, zero_shapes, zero_dtypes


def _ensure_compiled():
    global _prog, _dispatch
    if _dispatch is None:
        _prog = _build_program()
        _dispatch = _build_dispatch(_prog)
    return _dispatch


def _dispatch_device(s_i, W_action, actions):
    """Start the device call; returns a jax future for e (2048, 16) f32."""
    sharded, in_names, zshapes, zdtypes = _ensure_compiled()
    prep = _ensure_casts()
    act32 = np.asarray(actions).astype(np.int32)
    s8, W8, oh = (np.asarray(x) for x in prep(s_i[:T], W_action, act32))
    feed = {"s": s8, "Wsh": W8, "oh": oh}
    zeros = [np.zeros(s, d) for s, d in zip(zshapes, zdtypes)]
    return sharded(*[feed[nm] for nm in in_names], *zeros)[0]


def _host_phase_a(s_i, W_stop, W_start):
    """Everything independent of the action head. Returns (G, const) with
    tot = sum_t log((e * G[t]).sum()) + const."""
    f32 = np.float32
    s = s_i.astype(f32)
    Wss = np.concatenate([W_stop.astype(f32), W_start.astype(f32)], axis=1)
    Z = s @ Wss                                   # (T+1, 48)
    delta = Z[:, 0:2 * B:2] - Z[:, 1:2 * B:2]     # (T+1, B) stop0 - stop1
    lsr = Z[:, 2 * B:2 * B + B]                   # (T+1, B)
    expm = np.exp(-delta)
    ds = (1.0 / (1.0 + expm)).astype(f32)
    ss = (expm * ds).astype(f32)
    ld = (-np.log1p(expm)).astype(f32)
    er = np.exp(lsr[:T])
    at = (np.exp(f32(-PEN)) * er / er.sum(-1, keepdims=True)).astype(f32)

    ld = ld.copy()
    ld[0] = 0.0
    C = np.cumsum(ld[:T], 0, dtype=f32)
    const = 0.0
    logscale = 0.0
    aux = []
    for c in range(NCHUNK):
        i0 = c * L
        Cl = C[i0:i0 + L]
        Cstart = C[i0 - 1] if c > 0 else np.zeros(B, f32)
        Cm = (0.5 * (Cstart + Cl[-1])).astype(f32)
        Clprev = np.vstack([Cstart, Cl[:-1]])
        alpha = ss[i0:i0 + L] * np.exp(Clprev - Cm)
        beta = at[i0:i0 + L] * np.exp(Cm - Cl)
        if c == 0:
            alpha[0] = 0.0
            beta[0] = 0.0
        aux.append((Cl, Cm, alpha, beta))
    # solve p = alpha@zhat + tril(alpha beta^T, -1) p per chunk by block
    # forward substitution on the rank-B structure: O(T*B^2) instead of the
    # former O(NCHUNK*L^3) doubling -- keeps phase A well under the device
    # call's flight time even on this 1-core box
    BL = 16                                       # rows per block
    NB = L // BL
    trilb = np.tril(np.ones((BL, BL), f32), -1)
    G = np.empty((T, B), f32)
    zend = None
    for c in range(NCHUNK):
        i0 = c * L
        Cl, Cm, alpha, beta = aux[c]
        if c == 0:
            e0 = np.exp(lsr[0])
            zhat = (e0 / e0.sum() * np.exp(Cm)).astype(f32)
        cvec = alpha @ zhat                       # (L,)
        p = np.empty(L, f32)
        S = np.zeros(B, f32)
        for j in range(NB):
            sl = slice(j * BL, (j + 1) * BL)
            a_j, b_j = alpha[sl], beta[sl]
            q = cvec[sl] + a_j @ S
            X = trilb * (a_j @ b_j.T)             # 16x16 strictly lower
            # exact (I-X)^-1 q for nilpotent X: (I+X)(I+X^2)(I+X^4)(I+X^8)
            for st in range(4):
                q = q + X @ q
                if st < 3:
                    X = X @ X
            p[sl] = q
            S = S + b_j.T @ q
        Y = zhat[None, :] + np.cumsum(beta * p[:, None], 0, dtype=f32)
        G[i0:i0 + L] = np.exp(Cl - Cm) * Y
        const += L * logscale
        zend = np.exp(Cl[-1] - Cm) * Y[-1]
        if c < NCHUNK - 1:
            mu = zend.sum()
            zhat = ((zend / mu) * np.exp(aux[c + 1][1] - Cl[-1])).astype(f32)
            logscale += np.log(mu)
    const += np.log((ds[T] * zend).sum()) + logscale
    return G, const


def kernel(s_i, W_action, W_stop, W_start, actions):
    s_i = np.asarray(s_i, np.float32)
    e_fut = _dispatch_device(s_i, np.asarray(W_action, np.float32), actions)
    G, const = _host_phase_a(s_i, W_stop, W_start)   # hidden under the call
    e = np.asarray(e_fut).astype(np.float32)          # (2048, 16) bf16->f32
    w = (e * G).sum(1)
    return np.float32(np.log(w).sum() + const)
